# revision 19
# baseline (speedup 1.0000x reference)
"""Trainium2 Bass kernel for nn_FDConv (per-sample frequency-domain-synthesized
3x3 grouped conv).

Strategy (data-parallel over batch, 1 sample per NeuronCore):
  - host: permute dft_weight into dense half-spectrum layout (pure gather),
    precompute DFT basis matrices as constants (tap-major column order so
    stage-2 needs only contiguous rhs slices), stage x as bf16 in a padded
    parity-split layout so every load descriptor is one contiguous run.
  - device per core:
      warm-up dummy matmul chain keeps the PE busy from the preamble end so
        the DVFS p-state ramps to full clock before real work arrives
      att = sigmoid(logits) on ACT into 97 replicated partitions
      stage 1: per-kernel-k spectra GT_k = D_k^T . [C | S] (PE, starts as
        DMA slices arrive, no att dependency)
      att mix: GT = sum_k att_k GT_k  (scalar_tensor_tensor on DVE/GpSimd,
        pipelined behind stage-1 k-completions)
      stage 2: six 128x128 conv weight mats via 24 matmuls (tap-major
        contiguous rhs slices)
      conv: x bf16, even rows on partitions 0-63, odd on 64-127; six
        K=128/M=128/N=512 matmuls per 2 output row pairs in PSUM (structural
        optimum for this PE: 75% MAC utilization).
  - outputs converted to bf16 on PSUM->SBUF copy (DVE/ACT) and DMA'd back
    bf16 (halves store traffic); host reassembles rows and upcasts to fp32.
"""

import numpy as np
import ml_dtypes

import concourse.bass as bass
import concourse.bacc as bacc
import concourse.tile as tile
import concourse.mybir as mybir
from concourse.bass_utils import run_bass_kernel_spmd

F32 = mybir.dt.float32
BF16 = mybir.dt.bfloat16

B, CIN, COUT, KS = 8, 64, 64, 3
H, W = 256, 256
KNUM = 4
D1, D2 = COUT * KS, CIN * KS          # 192, 192
D2R = D2 // 2 + 1                     # 97
NF = D1 * D2R                         # 18624

NPAIR = 128          # output row pairs (2u+1, 2u+2), u = -1..127
SLOT = W + 2         # 258: [pad, 256 cols, pad] per row-slot
CHS = 16             # slots per x chunk
NCH = NPAIR // CHS   # 8 chunks

DDW = 4 * D2R        # 388: per-k dd slice [dre_h0|dre_h1|dim_h0|dim_h1]


def _host_constants():
    fh = np.fft.fftfreq(D1)
    fw = np.fft.rfftfreq(D2)
    dist = np.sqrt(fh[:, None] ** 2 + fw[None, :] ** 2)
    idx = np.argsort(dist.ravel(), kind='stable')
    FH = (idx // D2R).astype(np.int64)
    FW = (idx % D2R).astype(np.int64)
    perm = FH * D2R + FW
    inv = np.empty(NF, dtype=np.int64)
    inv[perm] = np.arange(NF)

    hh = np.arange(D1)
    ang = 2.0 * np.pi * np.outer(hh, hh) / D1
    # att scale 2/KNUM = 0.5 folded into the stage-1 basis
    Cb = (np.cos(ang) * (0.5 / D1)).astype(np.float32)   # [h, d1]
    Sb = (np.sin(ang) * (0.5 / D1)).astype(np.float32)
    # tap-major reversed column order: d1' = (2 - rtap)*64 + co so that
    # stage-2 rhs blocks [tap2|tap1|tap0] are contiguous 64-col slices
    permc = np.empty(D1, dtype=np.int64)
    for rtap in range(3):
        permc[(2 - rtap) * 64: (3 - rtap) * 64] = np.arange(64) * 3 + rtap
    Cb = Cb[:, permc]
    Sb = Sb[:, permc]
    # cs pack: [96, 6*192]: C2a C2b | S2a S2b | S2na S2nb  (a = spectrum rows
    # 0-95 on partitions, b = rows 96-191)
    cs = np.concatenate(
        [Cb[:96], Cb[96:], Sb[:96], Sb[96:], -Sb[:96], -Sb[96:]], axis=1
    ).astype(ml_dtypes.bfloat16)                          # [96, 1152]

    w_ = np.arange(D2R)
    n_ = np.arange(D2)
    alpha = np.full(D2R, 2.0); alpha[0] = 1.0; alpha[-1] = 1.0
    beta = np.full(D2R, 2.0); beta[0] = 0.0; beta[-1] = 0.0
    ang2 = 2.0 * np.pi * np.outer(w_, n_) / D2
    A = (alpha[:, None] * np.cos(ang2) / D2).astype(np.float32)   # [97, 192]
    Bm = (beta[:, None] * np.sin(ang2) / D2).astype(np.float32)
    ab = np.concatenate(
        [A[:, dx::3] for dx in range(3)] + [-Bm[:, dx::3] for dx in range(3)],
        axis=1,
    ).astype(ml_dtypes.bfloat16)                          # [97, 384]
    return inv, cs, ab


_INV, _CS, _AB = _host_constants()

# (which, dx) order used in the conv weight loop; t_sb index = 2*dx + which
_WSEQ = [(0, 0), (1, 0), (0, 1), (1, 1), (0, 2), (1, 2)]
# stage-2 per `which`: list of (Jblock, out_col0, out_ncols, rhs_col0)
# rhs cols in tap-major-reversed gt layout ([tap2|tap1|tap0] blocks of 64)
_S2MM = {
    0: [(0, 0, 64, 128),      # (J0 -> r0) tap0
        (1, 0, 128, 64)],     # (J1 -> r0 tap1, r1 tap0)
    1: [(0, 0, 128, 0),       # (J0 -> r0 tap2, r1 tap1)
        (1, 64, 64, 0)],      # (J1 -> r1) tap2
}
_ZQUAD = {0: (0, 1), 1: (1, 0)}  # zero quadrant (J, r)


def _emit_kernel(tc):
    nc = tc.nc
    from contextlib import ExitStack

    # x: [parity, cin, slot, 258] bf16, host-padded (col 0 and 257 are zeros)
    x_in = nc.dram_tensor("x_in", [2, CIN, NPAIR, SLOT], BF16,
                          kind="ExternalInput").ap()
    lg_in = nc.dram_tensor("lg_in", [1, KNUM], F32, kind="ExternalInput").ap()
    k_in = nc.dram_tensor("k_in", [96, 6 * D1 + KNUM * DDW], BF16,
                          kind="ExternalInput").ap()
    ab_in = nc.dram_tensor("ab_in", [D2R, 6 * 64], BF16, kind="ExternalInput").ap()
    # device-side output layout: plane r=0 slot s = row 2s-1, plane r=1 slot
    # s = row 2s. Keeps every store descriptor contiguous (multi-KB) per
    # partition; host reassembles the row interleave.
    out = nc.dram_tensor("out", [2, COUT, NPAIR + 1, W], BF16,
                         kind="ExternalOutput").ap()

    with ExitStack() as ctx:
        cpool = ctx.enter_context(tc.tile_pool(name="cpool", bufs=1))
        xbpool = ctx.enter_context(tc.tile_pool(name="xbpool", bufs=8))
        spool = ctx.enter_context(tc.tile_pool(name="spool", bufs=3))

        # ---- warm-up scratch (memset on vector so descgen engines stay free)
        scr_sb = cpool.tile([128, 64], BF16, name="scr_sb")
        nc.vector.memset(scr_sb[:], 0.0)

        # ---- DMA descriptors. Each ring only keeps ~4-8 packets in
        # flight, so per-ring throughput is the startup limiter: the
        # critical pack is split across ALL THREE rings by partition
        # range, and the single-packet logits ride ahead of everything
        # on the sync ring (a sem behind a big desc dribbles in-ring).
        lg_sb = cpool.tile([1, KNUM], F32, name="lg_sb")
        nc.sync.dma_start(out=lg_sb[:], in_=lg_in)
        k_sb = cpool.tile([96, 6 * D1 + KNUM * DDW], BF16, name="k_sb")
        for (p0, p1), eng in (((0, 32), nc.gpsimd), ((32, 64), nc.sync),
                              ((64, 96), nc.scalar)):
            eng.dma_start(out=k_sb[p0:p1, :], in_=k_in[p0:p1, :])
        cs_sb = k_sb[:, 0:6 * D1]
        dd_sb = k_sb[:, 6 * D1:]
        sig_sb = cpool.tile([1, KNUM], F32, name="sig_sb")
        nc.scalar.activation(sig_sb[:], lg_sb[:],
                             mybir.ActivationFunctionType.Sigmoid)
        ones_sb = cpool.tile([1, 128], F32, name="ones_sb")
        nc.vector.memset(ones_sb[:], 1.0)
        ab_sb = cpool.tile([D2R, 6 * 64], BF16, name="ab_sb")
        nc.scalar.dma_start(out=ab_sb[:], in_=ab_in)

        # ---- x chunk loads: slot t holds rows (2t, 2t+1). Chunk 0 early on
        # the sync ring (alone); chunk 1 gated on stage-1 start; chunks >=2
        # gated on the dft chain finishing.
        xch = []

        def load_xchunk(c, engs, anchor=None):
            # chunks hold 17 slots (1-slot overlap) so 2-pair windows never
            # cross a tile boundary; the last chunk has no slot 128
            nsl = CHS + 1 if c + 1 < NCH else CHS
            ne = len(engs)
            for i in range(2 * ne):
                par, (q0, q1) = i // ne, (
                    (i % ne) * 64 // ne, (i % ne + 1) * 64 // ne)
                di = engs[i % ne].dma_start(
                    out=xch[c][64 * par + q0: 64 * par + q1, 0:nsl * SLOT],
                    in_=x_in[par, q0:q1, c * CHS:c * CHS + nsl, :])
                if anchor is not None:
                    bass._add_dep_helper(
                        di.ins, anchor.ins,
                        reason="x chunks yield DMA to the dft criticals")

        for c in range(NCH):
            xch.append(xbpool.tile([128, (CHS + 1) * SLOT], BF16, name="xb"))

        def slot_rhs(s, dx, npair=1):
            # [128, npair, W] window starting at slot s (npair<=2; both slots
            # live in chunk s//CHS thanks to the 1-slot overlap)
            c, loc = s // CHS, s % CHS
            if npair == 1:
                return xch[c][:, loc * SLOT + dx: loc * SLOT + dx + W]
            v = xch[c].rearrange("p (t s) -> p t s", s=SLOT)
            return v[:, loc:loc + npair, dx:dx + W]

        gtre_sb = cpool.tile([D2R, D1], BF16, name="gtre_sb")
        gtim_sb = cpool.tile([D2R, D1], BF16, name="gtim_sb")
        mxre_sb = [cpool.tile([D2R, D1], F32, name=f"mxre_{i}") for i in range(2)]
        mxim_sb = [cpool.tile([D2R, D1], F32, name=f"mxim_{i}") for i in range(2)]
        t_sb = [cpool.tile([128, 128], BF16, name=f"t_sb_{i}") for i in range(6)]

        with tc.tile_pool(name="dftps", bufs=1, space="PSUM") as dpool:
            # one PSUM bank shared by the warm-up dummies (cols 4:68) and
            # the att broadcast (cols 0:4) -- PSUM allocation is bank-granular
            warm_ps = dpool.tile([128, 68], F32, name="warm_ps")

            def dummies(n):
                for _ in range(n):
                    nc.tensor.matmul(warm_ps[0:64, 4:68], scr_sb[:, 0:64],
                                     scr_sb[:, 0:64], start=True, stop=True)

            # PE busy from preamble end; DVFS hits full clock ~3us later,
            # right as the first DMA-gated stage-1 matmul becomes ready
            # (Tile's scheduler packs all ready work first, so one block)
            dummies(75)

            # att broadcast to 97 partitions via a K=1 matmul (scheduled
            # into the dummy stream once sigmoid lands)
            nc.tensor.matmul(warm_ps[:, 0:KNUM], ones_sb[:], sig_sb[:],
                             start=True, stop=True)
            att_sb = cpool.tile([D2R, KNUM], F32, name="att_sb")
            nc.vector.tensor_copy(att_sb[:], warm_ps[0:D2R, 0:KNUM])

            # ---- stage 1: per-k half-spectrum iFFT along axis 0
            # GTre_k = dre_k0^T C2a + dre_k1^T C2b + dim_k0^T S2na + dim_k1^T S2nb
            # GTim_k = dre_k0^T S2a + dre_k1^T S2b + dim_k0^T C2a + dim_k1^T C2b
            gt_ps = [dpool.tile([D2R, 2 * D1], F32, name=f"gt_ps_{i}")
                     for i in range(4)]   # re_k01, re_k23, im_k01, im_k23

            def gtv(tgt, k):
                return gt_ps[2 * tgt + k // 2][:, (k % 2) * D1:(k % 2 + 1) * D1]

            O_C, O_S, O_SN = 0, 2 * D1, 4 * D1
            k0_anchor = k_last = None
            for k in range(KNUM):
                dre = [dd_sb[:, k * DDW + h * D2R: k * DDW + (h + 1) * D2R]
                       for h in range(2)]
                dim = [dd_sb[:, k * DDW + (2 + h) * D2R: k * DDW + (3 + h) * D2R]
                       for h in range(2)]
                for tgt, (bre, bim) in enumerate(((O_C, O_SN), (O_S, O_C))):
                    o = gtv(tgt, k)
                    mi = nc.tensor.matmul(o, dre[0], cs_sb[:, bre:bre + D1],
                                          start=True, stop=False)
                    nc.tensor.matmul(o, dre[1], cs_sb[:, bre + D1:bre + 2 * D1],
                                     start=False, stop=False)
                    nc.tensor.matmul(o, dim[0], cs_sb[:, bim:bim + D1],
                                     start=False, stop=False)
                    nc.tensor.matmul(o, dim[1], cs_sb[:, bim + D1:bim + 2 * D1],
                                     start=False, stop=True)
                    if k0_anchor is None:
                        k0_anchor = mi
                    k_last = mi
            # chunk 0 transfers right after the critical pack clears the
            # engines; chunk 1 after stage-1 wraps up
            load_xchunk(0, [nc.sync, nc.gpsimd, nc.scalar], anchor=k0_anchor)
            load_xchunk(1, [nc.sync, nc.gpsimd], anchor=k_last)

            # ---- att mix (pipelined behind stage-1): re chain on DVE;
            # im multiplies on ACT (per-partition scale), im adds on DVE
            # (gpsimd cannot read PSUM)
            MU, AD = mybir.AluOpType.mult, mybir.AluOpType.add
            nc.vector.tensor_scalar_mul(mxre_sb[0][:], gtv(0, 0),
                                        att_sb[:, 0:1])
            for k in (1, 2):
                nc.vector.scalar_tensor_tensor(
                    mxre_sb[k % 2][:], gtv(0, k), att_sb[:, k:k + 1],
                    mxre_sb[(k + 1) % 2][:], MU, AD)
            nc.vector.scalar_tensor_tensor(
                gtre_sb[:], gtv(0, 3), att_sb[:, 3:4], mxre_sb[0][:], MU, AD)
            pim = [cpool.tile([D2R, D1], F32, name=f"pim_{k}") for k in range(4)]
            for k in range(4):
                nc.scalar.mul(pim[k][:], gtv(1, k), att_sb[:, k:k + 1])
            nc.vector.scalar_tensor_tensor(
                mxim_sb[0][:], pim[1][:], 1.0, pim[0][:], MU, AD)
            nc.vector.scalar_tensor_tensor(
                mxim_sb[1][:], pim[2][:], 1.0, mxim_sb[0][:], MU, AD)
            nc.vector.scalar_tensor_tensor(
                gtim_sb[:], pim[3][:], 1.0, mxim_sb[1][:], MU, AD)

            # ---- stage 2: six conv weight matrices T[(J,ci),(r,co)]
            t_copy = t0_copy = None
            for i, (which, dx) in enumerate(_WSEQ):
                t_ps = dpool.tile([128, 128], F32, name="t_ps", bufs=3)
                zj, zr = _ZQUAD[which]
                nc.vector.memset(t_ps[64 * zj:64 * zj + 64,
                                      64 * zr:64 * zr + 64], 0.0)
                for (J, c0, ncol, r0) in _S2MM[which]:
                    o = t_ps[64 * J:64 * J + 64, c0:c0 + ncol]
                    nc.tensor.matmul(o, ab_sb[:, dx * 64:(dx + 1) * 64],
                                     gtre_sb[:, r0:r0 + ncol],
                                     start=True, stop=False)
                    nc.tensor.matmul(o, ab_sb[:, (3 + dx) * 64:(4 + dx) * 64],
                                     gtim_sb[:, r0:r0 + ncol],
                                     start=False, stop=True)
                if i % 2 == 0:
                    t_copy = nc.vector.tensor_copy(t_sb[2 * dx + which][:],
                                                   t_ps[:])
                else:
                    t_copy = nc.scalar.copy(t_sb[2 * dx + which][:], t_ps[:])
                if t0_copy is None:
                    t0_copy = t_copy

        # chunks 2-3 enter the gpsimd ring once the dft chain is done;
        # chunks 4-7 are released two conv blocks ahead of their use
        load_xchunk(2, [nc.gpsimd], anchor=t0_copy)
        load_xchunk(3, [nc.gpsimd], anchor=t0_copy)

        # ---- conv over row pairs
        # staging groups over pair slots s = u+1 in [0, 129): big early, small
        # at the end so the last stores drain quickly
        gsizes = [16] * 7 + [8, 4, 2, 2, 1]
        gstart = np.cumsum([0] + gsizes).tolist()   # [0,16,...,112,120,124,126,128,129]

        def group_of(s):
            for gi in range(len(gsizes)):
                if s < gstart[gi + 1]:
                    return gi, s - gstart[gi]
            raise AssertionError

        # units: (-1,) special, (0,1), (2,3), ..., (124,125), (126,), (127,)
        units = [(-1,)] + [(u, u + 1) for u in range(0, 126, 2)] + [(126,), (127,)]

        with tc.tile_pool(name="convps", bufs=8, space="PSUM") as cps:
            staging = {}

            def get_staging(gi):
                if gi not in staging:
                    if gi >= 7:
                        # small late groups get dedicated slots so the final
                        # copies never wait on store completions
                        staging[gi] = spool.tile(
                            [128, gsizes[gi] * W], BF16,
                            name=f"staging_l{gi}", bufs=1)
                    else:
                        staging[gi] = spool.tile(
                            [128, gsizes[gi] * W], BF16, name="staging")
                return staging[gi]

            def unit_mms(un):
                L = []
                for wh, dx in _WSEQ:
                    if wh == 0 and un[0] < 0:
                        continue
                    if wh == 1 and un[0] > 126:
                        continue
                    L.append((wh, dx))
                return L

            def emit_block(uns):
                tiles = {}
                last_mm = None
                for un in uns:
                    tiles[un] = cps.tile([128, len(un) * W], F32, name="pair_ps")
                plan = {un: unit_mms(un) for un in uns}
                for k, (wh, dx) in enumerate(_WSEQ):
                    for un in uns:
                        if (wh, dx) not in plan[un]:
                            continue
                        i = plan[un].index((wh, dx))
                        rhs = slot_rhs(un[0] + (0 if wh == 0 else 1), dx,
                                       len(un))
                        last_mm = nc.tensor.matmul(
                            tiles[un][:], t_sb[2 * dx + wh][:], rhs,
                            start=(i == 0), stop=(i == len(plan[un]) - 1),
                            skip_group_check=True)
                for un in uns:
                    for j, u in enumerate(un):
                        gi, si = group_of(u + 1)
                        st = get_staging(gi)[:, si * W:(si + 1) * W]
                        src = tiles[un][:, j * W:(j + 1) * W]
                        if u == -1:
                            nc.scalar.copy(st[64:128, :], src[64:128, :])
                        elif u == 127:
                            nc.scalar.copy(st[0:64, :], src[0:64, :])
                        elif (j == 0 and len(un) == 2 and
                              group_of(un[1] + 1)[0] == gi):
                            # both halves land in the same staging tile: one
                            # wide copy, alternating engines per unit
                            st2 = get_staging(gi)[:, si * W:(si + 2) * W]
                            if (u // 2) % 2 == 0:
                                nc.vector.tensor_copy(st2, tiles[un][:])
                            else:
                                nc.scalar.copy(st2, tiles[un][:])
                            break
                        elif u % 2 == 0:
                            nc.vector.tensor_copy(st, src)
                        else:
                            nc.scalar.copy(st, src)
                    for u in un:
                        gi, si = group_of(u + 1)
                        if si == gsizes[gi] - 1:
                            emit_stores(gi)
                return last_mm

            store_cnt = [0]

            def store_dma(dst, src):
                engs = [nc.gpsimd, nc.sync]
                eng = engs[store_cnt[0] % len(engs)]
                store_cnt[0] += 1
                eng.dma_start(out=dst, in_=src)

            def emit_stores(gi):
                stg = staging.pop(gi)
                s0, s1 = gstart[gi], gstart[gi + 1]
                sv = stg.rearrange("p (g w) -> p g w", w=W)
                if gi == 0:
                    # row 0 from pair u=-1 -> plane 1, slot 0
                    store_dma(out[1, :, 0:1, :], sv[64:128, 0:1, :])
                # full pairs in this group: slots max(s0,1) .. min(s1,128)-1
                fa, fb = max(s0, 1), min(s1, 128)
                run = 8 if gi < 7 else 4
                va = fa
                while va < fb:
                    vb = min(fb, va + run)
                    G = vb - va
                    store_dma(out[0, :, va:vb, :],
                              sv[0:64, va - s0:va - s0 + G, :])
                    store_dma(out[1, :, va:vb, :],
                              sv[64:128, va - s0:va - s0 + G, :])
                    va = vb
                if s1 == 129:
                    # row 255 from pair u=127 -> plane 0, slot 128
                    store_dma(out[0, :, NPAIR:NPAIR + 1, :],
                              sv[0:64, 128 - s0:129 - s0, :])

            # blocks of up to 4 units; chunks 4-7 released well ahead of
            # their first consuming block
            ui = 0
            bi = 0
            while ui < len(units):
                blk_mm = emit_block(units[ui:ui + 4])
                if bi == 1:
                    for c in range(4, NCH):
                        load_xchunk(c, [nc.gpsimd], anchor=blk_mm)
                ui += 4
                bi += 1


_NC_CACHE = None


def _build_nc():
    global _NC_CACHE
    if _NC_CACHE is None:
        nc = bacc.Bacc("TRN2", target_bir_lowering=False, debug=False,
                       num_devices=B)
        with tile.TileContext(nc) as tc:
            _emit_kernel(tc)
        nc.compile()
        _NC_CACHE = nc
    return _NC_CACHE


def _in_maps(x, k_att_logits, dft_weight):
    x = np.asarray(x, dtype=np.float32)
    lg = np.asarray(k_att_logits, dtype=np.float32)
    dw = np.asarray(dft_weight, dtype=np.float32)

    # x -> bf16, parity-split rows, host-inserted zero pad columns
    xp = np.zeros((B, 2, CIN, NPAIR, SLOT), dtype=ml_dtypes.bfloat16)
    xv = x.reshape(B, CIN, NPAIR, 2, W).transpose(0, 3, 1, 2, 4)  # [b,j,c,t,w]
    xp[:, :, :, :, 1:1 + W] = xv.astype(ml_dtypes.bfloat16)

    # host-side gather: dense half-spectrum layout [k, half, p, w, c]
    dftP = dw[:, _INV, :].reshape(KNUM, 2, 96, D2R, 2)
    # per-k slice [96, 4*97]: dre_h0 | dre_h1 | dim_h0 | dim_h1, k-major
    dd = np.concatenate(
        [np.concatenate([dftP[k, 0, :, :, 0], dftP[k, 1, :, :, 0],
                         dftP[k, 0, :, :, 1], dftP[k, 1, :, :, 1]], axis=1)
         for k in range(KNUM)], axis=1).astype(ml_dtypes.bfloat16)
    kin = np.ascontiguousarray(np.concatenate([_CS, dd], axis=1))

    maps = []
    for b in range(B):
        maps.append({
            "x_in": np.ascontiguousarray(xp[b]),
            "lg_in": lg[b:b + 1],
            "k_in": kin,
            "ab_in": _AB,
        })
    return maps


def _execute(x, k_att_logits, dft_weight, trace=False, **trace_kwargs):
    nc = _build_nc()
    res = run_bass_kernel_spmd(
        nc, _in_maps(x, k_att_logits, dft_weight),
        core_ids=list(range(B)), trace=trace, **trace_kwargs)
    out = np.empty((B, COUT, H, W), dtype=np.float32)
    for b in range(B):
        dev = res.results[b]["out"].astype(np.float32)  # [2, COUT, NPAIR+1, W]
        out[b, :, 1::2, :] = dev[0, :, 1:NPAIR + 1, :]
        out[b, :, 0::2, :] = dev[1, :, 0:NPAIR, :]
    return out, res


def kernel(x, k_att_logits, dft_weight):
    out, _ = _execute(x, k_att_logits, dft_weight)
    return out.astype(np.float32)


# revision 21
# speedup vs baseline: 1.0233x; 1.0233x over previous
"""Trainium2 Bass kernel for nn_FDConv (per-sample frequency-domain-synthesized
3x3 grouped conv).

Strategy (data-parallel over batch, 1 sample per NeuronCore):
  - host: permute dft_weight into dense half-spectrum layout (pure gather),
    precompute DFT basis matrices as constants (tap-major column order so
    stage-2 needs only contiguous rhs slices), stage x as bf16 in a padded
    parity-split layout so every load descriptor is one contiguous run.
  - device per core:
      warm-up dummy matmul chain keeps the PE busy from the preamble end so
        the DVFS p-state ramps to full clock before real work arrives
      att = sigmoid(logits) on ACT into 97 replicated partitions
      stage 1: per-kernel-k spectra GT_k = D_k^T . [C | S] (PE, starts as
        DMA slices arrive, no att dependency)
      att mix: GT = sum_k att_k GT_k  (scalar_tensor_tensor on DVE/GpSimd,
        pipelined behind stage-1 k-completions)
      stage 2: six 128x128 conv weight mats via 24 matmuls (tap-major
        contiguous rhs slices)
      conv: x bf16, even rows on partitions 0-63, odd on 64-127; six
        K=128/M=128/N=512 matmuls per 2 output row pairs in PSUM (structural
        optimum for this PE: 75% MAC utilization).
  - outputs converted to bf16 on PSUM->SBUF copy (DVE/ACT) and DMA'd back
    bf16 (halves store traffic); host reassembles rows and upcasts to fp32.
"""

import numpy as np
import ml_dtypes

import concourse.bass as bass
import concourse.bacc as bacc
import concourse.tile as tile
import concourse.mybir as mybir
from concourse.bass_utils import run_bass_kernel_spmd

F32 = mybir.dt.float32
BF16 = mybir.dt.bfloat16

B, CIN, COUT, KS = 8, 64, 64, 3
H, W = 256, 256
KNUM = 4
D1, D2 = COUT * KS, CIN * KS          # 192, 192
D2R = D2 // 2 + 1                     # 97
NF = D1 * D2R                         # 18624

NPAIR = 128          # output row pairs (2u+1, 2u+2), u = -1..127
SLOT = W + 2         # 258: [pad, 256 cols, pad] per row-slot
CHS = 16             # slots per x chunk
NCH = NPAIR // CHS   # 8 chunks

DDW = 4 * D2R        # 388: per-k dd slice [dre_h0|dre_h1|dim_h0|dim_h1]


def _host_constants():
    fh = np.fft.fftfreq(D1)
    fw = np.fft.rfftfreq(D2)
    dist = np.sqrt(fh[:, None] ** 2 + fw[None, :] ** 2)
    idx = np.argsort(dist.ravel(), kind='stable')
    FH = (idx // D2R).astype(np.int64)
    FW = (idx % D2R).astype(np.int64)
    perm = FH * D2R + FW
    inv = np.empty(NF, dtype=np.int64)
    inv[perm] = np.arange(NF)

    hh = np.arange(D1)
    ang = 2.0 * np.pi * np.outer(hh, hh) / D1
    # att scale 2/KNUM = 0.5 folded into the stage-1 basis
    Cb = (np.cos(ang) * (0.5 / D1)).astype(np.float32)   # [h, d1]
    Sb = (np.sin(ang) * (0.5 / D1)).astype(np.float32)
    # tap-major reversed column order: d1' = (2 - rtap)*64 + co so that
    # stage-2 rhs blocks [tap2|tap1|tap0] are contiguous 64-col slices
    permc = np.empty(D1, dtype=np.int64)
    for rtap in range(3):
        permc[(2 - rtap) * 64: (3 - rtap) * 64] = np.arange(64) * 3 + rtap
    Cb = Cb[:, permc]
    Sb = Sb[:, permc]
    # cs pack: [96, 6*192]: C2a C2b | S2a S2b | S2na S2nb  (a = spectrum rows
    # 0-95 on partitions, b = rows 96-191)
    cs = np.concatenate(
        [Cb[:96], Cb[96:], Sb[:96], Sb[96:], -Sb[:96], -Sb[96:]], axis=1
    ).astype(ml_dtypes.bfloat16)                          # [96, 1152]

    w_ = np.arange(D2R)
    n_ = np.arange(D2)
    alpha = np.full(D2R, 2.0); alpha[0] = 1.0; alpha[-1] = 1.0
    beta = np.full(D2R, 2.0); beta[0] = 0.0; beta[-1] = 0.0
    ang2 = 2.0 * np.pi * np.outer(w_, n_) / D2
    A = (alpha[:, None] * np.cos(ang2) / D2).astype(np.float32)   # [97, 192]
    Bm = (beta[:, None] * np.sin(ang2) / D2).astype(np.float32)
    ab = np.concatenate(
        [A[:, dx::3] for dx in range(3)] + [-Bm[:, dx::3] for dx in range(3)],
        axis=1,
    ).astype(ml_dtypes.bfloat16)                          # [97, 384]
    return inv, cs, ab


_INV, _CS, _AB = _host_constants()

# (which, dx) order used in the conv weight loop; t_sb index = 2*dx + which
_WSEQ = [(0, 0), (1, 0), (0, 1), (1, 1), (0, 2), (1, 2)]
# stage-2 per `which`: list of (Jblock, out_col0, out_ncols, rhs_col0)
# rhs cols in tap-major-reversed gt layout ([tap2|tap1|tap0] blocks of 64)
_S2MM = {
    0: [(0, 0, 64, 128),      # (J0 -> r0) tap0
        (1, 0, 128, 64)],     # (J1 -> r0 tap1, r1 tap0)
    1: [(0, 0, 128, 0),       # (J0 -> r0 tap2, r1 tap1)
        (1, 64, 64, 0)],      # (J1 -> r1) tap2
}
_ZQUAD = {0: (0, 1), 1: (1, 0)}  # zero quadrant (J, r)


def _emit_kernel(tc):
    nc = tc.nc
    from contextlib import ExitStack

    # x: [parity, cin, slot, 258] bf16, host-padded (col 0 and 257 are zeros)
    x_in = nc.dram_tensor("x_in", [2, CIN, NPAIR, SLOT], BF16,
                          kind="ExternalInput").ap()
    lg_in = nc.dram_tensor("lg_in", [1, KNUM], F32, kind="ExternalInput").ap()
    k_in = nc.dram_tensor("k_in", [96, 6 * D1 + KNUM * DDW], BF16,
                          kind="ExternalInput").ap()
    ab_in = nc.dram_tensor("ab_in", [D2R, 6 * 64], BF16, kind="ExternalInput").ap()
    # device-side output layout: plane r=0 slot s = row 2s-1, plane r=1 slot
    # s = row 2s. Keeps every store descriptor contiguous (multi-KB) per
    # partition; host reassembles the row interleave.
    out = nc.dram_tensor("out", [2, COUT, NPAIR + 1, W], BF16,
                         kind="ExternalOutput").ap()

    with ExitStack() as ctx:
        cpool = ctx.enter_context(tc.tile_pool(name="cpool", bufs=1))
        xbpool = ctx.enter_context(tc.tile_pool(name="xbpool", bufs=8))
        spool = ctx.enter_context(tc.tile_pool(name="spool", bufs=3))

        # ---- warm-up scratch (memset on vector so descgen engines stay free)
        scr_sb = cpool.tile([128, 64], BF16, name="scr_sb")
        nc.vector.memset(scr_sb[:], 0.0)

        # ---- DMA descriptors. Each ring only keeps ~4-8 packets in
        # flight, so per-ring throughput is the startup limiter: the
        # critical pack is split across ALL THREE rings by partition
        # range, and the single-packet logits ride ahead of everything
        # on the sync ring (a sem behind a big desc dribbles in-ring).
        lg_sb = cpool.tile([1, KNUM], F32, name="lg_sb")
        nc.sync.dma_start(out=lg_sb[:], in_=lg_in)
        k_sb = cpool.tile([96, 6 * D1 + KNUM * DDW], BF16, name="k_sb")
        for (p0, p1), eng in (((0, 32), nc.gpsimd), ((32, 64), nc.sync),
                              ((64, 96), nc.scalar)):
            eng.dma_start(out=k_sb[p0:p1, :], in_=k_in[p0:p1, :])
        cs_sb = k_sb[:, 0:6 * D1]
        dd_sb = k_sb[:, 6 * D1:]
        sig_sb = cpool.tile([1, KNUM], F32, name="sig_sb")
        nc.scalar.activation(sig_sb[:], lg_sb[:],
                             mybir.ActivationFunctionType.Sigmoid)
        ones_sb = cpool.tile([1, 128], F32, name="ones_sb")
        nc.vector.memset(ones_sb[:], 1.0)
        ab_sb = cpool.tile([D2R, 6 * 64], BF16, name="ab_sb")
        nc.scalar.dma_start(out=ab_sb[:], in_=ab_in)

        # ---- x chunk loads: slot t holds rows (2t, 2t+1). Chunk 0 early on
        # the sync ring (alone); chunk 1 gated on stage-1 start; chunks >=2
        # gated on the dft chain finishing.
        xch = []

        def load_xchunk(c, engs, anchor=None):
            # chunks hold 17 slots (1-slot overlap) so 2-pair windows never
            # cross a tile boundary; the last chunk has no slot 128.
            # Rings round-robin their in-flight descriptors, so chunk loads
            # are serialized per ring via anchors; returns last desc.
            nsl = CHS + 1 if c + 1 < NCH else CHS
            ne = len(engs)
            descs = []
            for par in range(2):
                for j in range(ne):
                    q0, q1 = j * 64 // ne, (j + 1) * 64 // ne
                    di = engs[j].dma_start(
                        out=xch[c][64 * par + q0: 64 * par + q1, 0:nsl * SLOT],
                        in_=x_in[par, q0:q1, c * CHS:c * CHS + nsl, :])
                    if anchor is not None:
                        bass._add_dep_helper(
                            di.ins, anchor.ins,
                            reason="x chunk serialized behind predecessor")
                    descs.append(di)
            return descs[-1]

        for c in range(NCH):
            xch.append(xbpool.tile([128, (CHS + 1) * SLOT], BF16, name="xb"))

        def slot_rhs(s, dx, npair=1):
            # [128, npair, W] window starting at slot s (npair<=2; both slots
            # live in chunk s//CHS thanks to the 1-slot overlap)
            c, loc = s // CHS, s % CHS
            if npair == 1:
                return xch[c][:, loc * SLOT + dx: loc * SLOT + dx + W]
            v = xch[c].rearrange("p (t s) -> p t s", s=SLOT)
            return v[:, loc:loc + npair, dx:dx + W]

        gtre_sb = cpool.tile([D2R, D1], BF16, name="gtre_sb")
        gtim_sb = cpool.tile([D2R, D1], BF16, name="gtim_sb")
        mxre_sb = [cpool.tile([D2R, D1], F32, name=f"mxre_{i}") for i in range(2)]
        mxim_sb = [cpool.tile([D2R, D1], F32, name=f"mxim_{i}") for i in range(2)]
        t_sb = [cpool.tile([128, 128], BF16, name=f"t_sb_{i}") for i in range(6)]

        with tc.tile_pool(name="dftps", bufs=1, space="PSUM") as dpool:
            # one PSUM bank shared by the warm-up dummies (cols 4:68) and
            # the att broadcast (cols 0:4) -- PSUM allocation is bank-granular
            warm_ps = dpool.tile([128, 68], F32, name="warm_ps")

            def dummies(n):
                for _ in range(n):
                    nc.tensor.matmul(warm_ps[0:64, 4:68], scr_sb[:, 0:64],
                                     scr_sb[:, 0:64], start=True, stop=True)

            # PE busy from preamble end; DVFS hits full clock ~3us later,
            # right as the first DMA-gated stage-1 matmul becomes ready
            # (Tile's scheduler packs all ready work first, so one block)
            dummies(110)

            # att broadcast to 97 partitions via a K=1 matmul (scheduled
            # into the dummy stream once sigmoid lands)
            nc.tensor.matmul(warm_ps[:, 0:KNUM], ones_sb[:], sig_sb[:],
                             start=True, stop=True)
            att_sb = cpool.tile([D2R, KNUM], F32, name="att_sb")
            nc.vector.tensor_copy(att_sb[:], warm_ps[0:D2R, 0:KNUM])

            # ---- stage 1: per-k half-spectrum iFFT along axis 0
            # GTre_k = dre_k0^T C2a + dre_k1^T C2b + dim_k0^T S2na + dim_k1^T S2nb
            # GTim_k = dre_k0^T S2a + dre_k1^T S2b + dim_k0^T C2a + dim_k1^T C2b
            gt_ps = [dpool.tile([D2R, 2 * D1], F32, name=f"gt_ps_{i}")
                     for i in range(4)]   # re_k01, re_k23, im_k01, im_k23

            def gtv(tgt, k):
                return gt_ps[2 * tgt + k // 2][:, (k % 2) * D1:(k % 2 + 1) * D1]

            O_C, O_S, O_SN = 0, 2 * D1, 4 * D1
            k0_anchor = k_last = None
            for k in range(KNUM):
                dre = [dd_sb[:, k * DDW + h * D2R: k * DDW + (h + 1) * D2R]
                       for h in range(2)]
                dim = [dd_sb[:, k * DDW + (2 + h) * D2R: k * DDW + (3 + h) * D2R]
                       for h in range(2)]
                for tgt, (bre, bim) in enumerate(((O_C, O_SN), (O_S, O_C))):
                    o = gtv(tgt, k)
                    mi = nc.tensor.matmul(o, dre[0], cs_sb[:, bre:bre + D1],
                                          start=True, stop=False)
                    nc.tensor.matmul(o, dre[1], cs_sb[:, bre + D1:bre + 2 * D1],
                                     start=False, stop=False)
                    nc.tensor.matmul(o, dim[0], cs_sb[:, bim:bim + D1],
                                     start=False, stop=False)
                    nc.tensor.matmul(o, dim[1], cs_sb[:, bim + D1:bim + 2 * D1],
                                     start=False, stop=True)
                    if k0_anchor is None:
                        k0_anchor = mi
                    k_last = mi
            # chunk 0 transfers right after the critical pack clears the
            # engines (sync+scalar rings); chunk 1 chains behind it
            c0_last = load_xchunk(0, [nc.sync, nc.scalar], anchor=k0_anchor)
            c1_last = load_xchunk(1, [nc.sync, nc.scalar], anchor=c0_last)

            # ---- att mix (pipelined behind stage-1): re chain on DVE;
            # im multiplies on ACT (per-partition scale), im adds on DVE
            # (gpsimd cannot read PSUM)
            MU, AD = mybir.AluOpType.mult, mybir.AluOpType.add
            nc.vector.tensor_scalar_mul(mxre_sb[0][:], gtv(0, 0),
                                        att_sb[:, 0:1])
            for k in (1, 2):
                nc.vector.scalar_tensor_tensor(
                    mxre_sb[k % 2][:], gtv(0, k), att_sb[:, k:k + 1],
                    mxre_sb[(k + 1) % 2][:], MU, AD)
            nc.vector.scalar_tensor_tensor(
                gtre_sb[:], gtv(0, 3), att_sb[:, 3:4], mxre_sb[0][:], MU, AD)
            pim = [cpool.tile([D2R, D1], F32, name=f"pim_{k}") for k in range(4)]
            for k in range(4):
                nc.scalar.mul(pim[k][:], gtv(1, k), att_sb[:, k:k + 1])
            nc.vector.scalar_tensor_tensor(
                mxim_sb[0][:], pim[1][:], 1.0, pim[0][:], MU, AD)
            nc.vector.scalar_tensor_tensor(
                mxim_sb[1][:], pim[2][:], 1.0, mxim_sb[0][:], MU, AD)
            nc.vector.scalar_tensor_tensor(
                gtim_sb[:], pim[3][:], 1.0, mxim_sb[1][:], MU, AD)

            # ---- stage 2: six conv weight matrices T[(J,ci),(r,co)]
            t_copy = t0_copy = None
            for i, (which, dx) in enumerate(_WSEQ):
                t_ps = dpool.tile([128, 128], F32, name="t_ps", bufs=3)
                zj, zr = _ZQUAD[which]
                nc.vector.memset(t_ps[64 * zj:64 * zj + 64,
                                      64 * zr:64 * zr + 64], 0.0)
                for (J, c0, ncol, r0) in _S2MM[which]:
                    o = t_ps[64 * J:64 * J + 64, c0:c0 + ncol]
                    nc.tensor.matmul(o, ab_sb[:, dx * 64:(dx + 1) * 64],
                                     gtre_sb[:, r0:r0 + ncol],
                                     start=True, stop=False)
                    nc.tensor.matmul(o, ab_sb[:, (3 + dx) * 64:(4 + dx) * 64],
                                     gtim_sb[:, r0:r0 + ncol],
                                     start=False, stop=True)
                if i % 2 == 0:
                    t_copy = nc.vector.tensor_copy(t_sb[2 * dx + which][:],
                                                   t_ps[:])
                else:
                    t_copy = nc.scalar.copy(t_sb[2 * dx + which][:], t_ps[:])
                if t0_copy is None:
                    t0_copy = t_copy

        # chunks 2+ chain one-at-a-time on the gpsimd ring behind the
        # critical pack's gpsimd slice
        prev = load_xchunk(2, [nc.gpsimd], anchor=t0_copy)
        prev = load_xchunk(3, [nc.gpsimd], anchor=prev)

        # ---- conv over row pairs
        # staging groups over pair slots s = u+1 in [0, 129): big early, small
        # at the end so the last stores drain quickly
        gsizes = [16] * 7 + [8, 4, 2, 2, 1]
        gstart = np.cumsum([0] + gsizes).tolist()   # [0,16,...,112,120,124,126,128,129]

        def group_of(s):
            for gi in range(len(gsizes)):
                if s < gstart[gi + 1]:
                    return gi, s - gstart[gi]
            raise AssertionError

        # units: (-1,) special, (0,1), (2,3), ..., (124,125), (126,), (127,)
        units = [(-1,)] + [(u, u + 1) for u in range(0, 126, 2)] + [(126,), (127,)]

        with tc.tile_pool(name="convps", bufs=8, space="PSUM") as cps:
            staging = {}

            def get_staging(gi):
                if gi not in staging:
                    if gi >= 7:
                        # small late groups get dedicated slots so the final
                        # copies never wait on store completions
                        staging[gi] = spool.tile(
                            [128, gsizes[gi] * W], BF16,
                            name=f"staging_l{gi}", bufs=1)
                    else:
                        staging[gi] = spool.tile(
                            [128, gsizes[gi] * W], BF16, name="staging")
                return staging[gi]

            def unit_mms(un):
                L = []
                for wh, dx in _WSEQ:
                    if wh == 0 and un[0] < 0:
                        continue
                    if wh == 1 and un[0] > 126:
                        continue
                    L.append((wh, dx))
                return L

            def emit_block(uns):
                tiles = {}
                last_mm = None
                for un in uns:
                    tiles[un] = cps.tile([128, len(un) * W], F32, name="pair_ps")
                plan = {un: unit_mms(un) for un in uns}
                for k, (wh, dx) in enumerate(_WSEQ):
                    for un in uns:
                        if (wh, dx) not in plan[un]:
                            continue
                        i = plan[un].index((wh, dx))
                        rhs = slot_rhs(un[0] + (0 if wh == 0 else 1), dx,
                                       len(un))
                        last_mm = nc.tensor.matmul(
                            tiles[un][:], t_sb[2 * dx + wh][:], rhs,
                            start=(i == 0), stop=(i == len(plan[un]) - 1),
                            skip_group_check=True)
                for un in uns:
                    for j, u in enumerate(un):
                        gi, si = group_of(u + 1)
                        st = get_staging(gi)[:, si * W:(si + 1) * W]
                        src = tiles[un][:, j * W:(j + 1) * W]
                        if u == -1:
                            nc.scalar.copy(st[64:128, :], src[64:128, :])
                        elif u == 127:
                            nc.scalar.copy(st[0:64, :], src[0:64, :])
                        elif (j == 0 and len(un) == 2 and
                              group_of(un[1] + 1)[0] == gi):
                            # both halves land in the same staging tile: one
                            # wide copy, alternating engines per unit
                            st2 = get_staging(gi)[:, si * W:(si + 2) * W]
                            if (u // 2) % 2 == 0:
                                nc.vector.tensor_copy(st2, tiles[un][:])
                            else:
                                nc.scalar.copy(st2, tiles[un][:])
                            break
                        elif u % 2 == 0:
                            nc.vector.tensor_copy(st, src)
                        else:
                            nc.scalar.copy(st, src)
                    for u in un:
                        gi, si = group_of(u + 1)
                        if si == gsizes[gi] - 1:
                            emit_stores(gi)
                return last_mm

            store_cnt = [0]

            def store_dma(dst, src):
                engs = [nc.sync, nc.scalar]
                eng = engs[store_cnt[0] % len(engs)]
                store_cnt[0] += 1
                eng.dma_start(out=dst, in_=src)

            def emit_stores(gi):
                stg = staging.pop(gi)
                s0, s1 = gstart[gi], gstart[gi + 1]
                sv = stg.rearrange("p (g w) -> p g w", w=W)
                if gi == 0:
                    # row 0 from pair u=-1 -> plane 1, slot 0
                    store_dma(out[1, :, 0:1, :], sv[64:128, 0:1, :])
                # full pairs in this group: slots max(s0,1) .. min(s1,128)-1
                fa, fb = max(s0, 1), min(s1, 128)
                run = 8 if gi < 7 else 4
                va = fa
                while va < fb:
                    vb = min(fb, va + run)
                    G = vb - va
                    store_dma(out[0, :, va:vb, :],
                              sv[0:64, va - s0:va - s0 + G, :])
                    store_dma(out[1, :, va:vb, :],
                              sv[64:128, va - s0:va - s0 + G, :])
                    va = vb
                if s1 == 129:
                    # row 255 from pair u=127 -> plane 0, slot 128
                    store_dma(out[0, :, NPAIR:NPAIR + 1, :],
                              sv[0:64, 128 - s0:129 - s0, :])

            # blocks of up to 4 units; chunks 4-7 released well ahead of
            # their first consuming block
            ui = 0
            bi = 0
            while ui < len(units):
                blk_mm = emit_block(units[ui:ui + 4])
                if bi == 1:
                    pv = blk_mm
                    for c in range(4, NCH):
                        pv = load_xchunk(c, [nc.gpsimd], anchor=pv)
                ui += 4
                bi += 1


_NC_CACHE = None


def _build_nc():
    global _NC_CACHE
    if _NC_CACHE is None:
        nc = bacc.Bacc("TRN2", target_bir_lowering=False, debug=False,
                       num_devices=B)
        with tile.TileContext(nc) as tc:
            _emit_kernel(tc)
        nc.compile()
        _NC_CACHE = nc
    return _NC_CACHE


def _in_maps(x, k_att_logits, dft_weight):
    x = np.asarray(x, dtype=np.float32)
    lg = np.asarray(k_att_logits, dtype=np.float32)
    dw = np.asarray(dft_weight, dtype=np.float32)

    # x -> bf16, parity-split rows, host-inserted zero pad columns
    xp = np.zeros((B, 2, CIN, NPAIR, SLOT), dtype=ml_dtypes.bfloat16)
    xv = x.reshape(B, CIN, NPAIR, 2, W).transpose(0, 3, 1, 2, 4)  # [b,j,c,t,w]
    xp[:, :, :, :, 1:1 + W] = xv.astype(ml_dtypes.bfloat16)

    # host-side gather: dense half-spectrum layout [k, half, p, w, c]
    dftP = dw[:, _INV, :].reshape(KNUM, 2, 96, D2R, 2)
    # per-k slice [96, 4*97]: dre_h0 | dre_h1 | dim_h0 | dim_h1, k-major
    dd = np.concatenate(
        [np.concatenate([dftP[k, 0, :, :, 0], dftP[k, 1, :, :, 0],
                         dftP[k, 0, :, :, 1], dftP[k, 1, :, :, 1]], axis=1)
         for k in range(KNUM)], axis=1).astype(ml_dtypes.bfloat16)
    kin = np.ascontiguousarray(np.concatenate([_CS, dd], axis=1))

    maps = []
    for b in range(B):
        maps.append({
            "x_in": np.ascontiguousarray(xp[b]),
            "lg_in": lg[b:b + 1],
            "k_in": kin,
            "ab_in": _AB,
        })
    return maps


def _execute(x, k_att_logits, dft_weight, trace=False, **trace_kwargs):
    nc = _build_nc()
    res = run_bass_kernel_spmd(
        nc, _in_maps(x, k_att_logits, dft_weight),
        core_ids=list(range(B)), trace=trace, **trace_kwargs)
    out = np.empty((B, COUT, H, W), dtype=np.float32)
    for b in range(B):
        dev = res.results[b]["out"].astype(np.float32)  # [2, COUT, NPAIR+1, W]
        out[b, :, 1::2, :] = dev[0, :, 1:NPAIR + 1, :]
        out[b, :, 0::2, :] = dev[1, :, 0:NPAIR, :]
    return out, res


def kernel(x, k_att_logits, dft_weight):
    out, _ = _execute(x, k_att_logits, dft_weight)
    return out.astype(np.float32)
